# revision 10
# baseline (speedup 1.0000x reference)
"""Trainium2 Bass kernel for nn_Block_3616362463321 (dense transformer block).

B=8, T=1024, C=1024, H=16, Dh=64. Data-parallel over batch: core b gets x[b].
Weights replicated to all 8 cores; no collectives.

Per-core layout strategy: activations live TRANSPOSED in SBUF ([C_part, T_free])
so every matmul consumes weights in natural [c_in, c_out] layout as lhsT and
activations as rhs, with no transposes inside the chain:
  - xn^T = LN1(x)^T           (stats via bn_stats on normal x tiles,
                               normalize in transposed space w/ broadcast rows,
                               split into independent t-halves so downstream
                               matmuls start on half 0 early)
  - Q^T, K^T = Wq/Wk^T chunks (lhsT=W[:, mchunk], rhs=xn^T)   [2 heads/chunk]
  - V natural = xn @ Wv       (lhsT=xn^T t-slice, rhs=Wv)  + ones column
  - S_h^T = K_h^T' @ Q_h^T    (K=64 contraction, 2 heads in row groups 0/64)
  - E = exp(S^T/8) with causal mask; blocks below the diagonal skipped
  - A_h^T|sums = [V_h|1]' @ E (M=65; fused softmax denominator in row 64)
  - attn^T = A^T / sums       (broadcast reciprocal via DRAM bounce)
  - y = attn^T' @ Wproj + x   (normal orientation; residual from reloaded x)
  - xn2^T = LN2(y)^T          (PE-transpose y, bn_stats on normal y)
  - h^T = relu(W1' @ xn2^T + b1)
  - out = h^T' @ W2 + y + b2  (normal orientation, DMA straight out)

All matmuls run float32r (TF32-like, 1 cycle/row at N>=256; measured rel err
~1.5e-4) with fp32 PSUM accumulation.  SBUF is managed as one arena pool whose
tag groups are reused across phase lifetimes; late-phase weights sit in groups
that die early so their DMAs prefetch during earlier compute.
"""
import sys

sys.path.insert(0, "/opt/trn_rl_repo")

from contextlib import ExitStack, nullcontext

import numpy as np

import concourse.bacc as bacc
import concourse.bass as bass
import concourse.mybir as mybir
import concourse.tile as tile
from concourse.bass_utils import run_bass_kernel_spmd
from concourse.masks import make_identity

P = 128
B, T, C, H = 8, 1024, 1024, 16
Dh = C // H            # 64
EPS = 1e-5
NF = 512               # matmul moving free dim (fp32 PSUM bank limit)
KC = C // P            # 8 c-chunks
TJ = T // P            # 8 t-chunks of 128
TN = T // NF           # 2 t-chunks of 512
F32 = mybir.dt.float32
F32R = mybir.dt.float32r
ALU = mybir.AluOpType
ACTF = mybir.ActivationFunctionType

N_CORES = 8

_CACHE = {}


def _bcast_row_ap(handle_ap, parts):
    """AP reading a [N]-shaped DRAM tensor broadcast across `parts` partitions."""
    return bass.AP(
        tensor=handle_ap.tensor,
        offset=handle_ap.offset,
        ap=[[0, parts], *handle_ap.ap],
    )


def build_nc(loop=1, hwloop=0):
    nc = bacc.Bacc("TRN2", target_bir_lowering=False, debug=False)

    x_d = nc.dram_tensor("x", [T, C], F32, kind="ExternalInput")
    wq_d = nc.dram_tensor("wq", [C, C], F32R, kind="ExternalInput")   # [c, (h d)]
    wk_d = nc.dram_tensor("wk", [C, C], F32R, kind="ExternalInput")
    wv_d = nc.dram_tensor("wv", [C, C], F32R, kind="ExternalInput")
    wp_d = nc.dram_tensor("wp", [C, C], F32R, kind="ExternalInput")   # [c_in, c_out]
    w1_d = nc.dram_tensor("w1", [C, C], F32R, kind="ExternalInput")
    w2_d = nc.dram_tensor("w2", [C, C], F32R, kind="ExternalInput")
    bp_d = nc.dram_tensor("bp", [C], F32, kind="ExternalInput")
    b1_d = nc.dram_tensor("b1", [C], F32, kind="ExternalInput")
    b2_d = nc.dram_tensor("b2", [C], F32, kind="ExternalInput")
    g1_d = nc.dram_tensor("g1", [C], F32, kind="ExternalInput")
    be1_d = nc.dram_tensor("be1", [C], F32, kind="ExternalInput")
    g2_d = nc.dram_tensor("g2", [C], F32, kind="ExternalInput")
    be2_d = nc.dram_tensor("be2", [C], F32, kind="ExternalInput")
    out_d = nc.dram_tensor("out", [T, C], F32, kind="ExternalOutput")

    # causal mask for a diagonal [s,t] block of S^T: keep where s <= t
    mask_np = np.where(
        np.arange(P)[:, None] <= np.arange(P)[None, :], 0.0, -1e9
    ).astype(np.float32)
    mask_c = nc.inline_tensor(mask_np, name="mask_const")

    with tile.TileContext(nc) as tc, ExitStack() as ES:
        singles = ES.enter_context(tc.tile_pool(name="singles", bufs=1))
        dram = ES.enter_context(tc.tile_pool(name="drsc", bufs=1, space="DRAM"))

        ident = singles.tile([P, P], F32)
        make_identity(nc, ident)
        maskS = singles.tile([P, P], F32)
        nc.sync.dma_start(out=maskS[:], in_=mask_c.ap())
        zeros1 = singles.tile([P, 1], F32)
        nc.vector.memset(zeros1[:], 0.0)
        epsc = singles.tile([P, 1], F32)
        nc.vector.memset(epsc[:], EPS)
        ones1 = singles.tile([P, 1], F32)
        nc.vector.memset(ones1[:], 1.0)
        # per-chunk columns: [P, KC] with element (p, k) = vec[k*P + p]
        cols = {}
        for nm, hd in (("g1", g1_d), ("be1", be1_d), ("g2", g2_d),
                       ("be2", be2_d), ("b1", b1_d)):
            t_ = singles.tile([P, KC], F32, tag=f"col_{nm}")
            nc.sync.dma_start(out=t_[:], in_=hd.ap().rearrange("(k p) -> p k", p=P))
            cols[nm] = t_
        bpb = singles.tile([P, C], F32)
        nc.sync.dma_start(out=bpb[:], in_=_bcast_row_ap(bp_d.ap(), P))
        b2b = singles.tile([P, C], F32)
        nc.sync.dma_start(out=b2b[:], in_=_bcast_row_ap(b2_d.ap(), P))

        # DRAM scratch for LN stat rows: one tile per t-half so each half's
        # broadcast only waits on its own 4 chunk writes
        ln_m = [[dram.tile([1, NF], F32, tag=f"lnm{i}_{h}", name=f"lnm{i}_{h}")
                 for h in range(TN)] for i in range(2)]
        ln_r = [[dram.tile([1, NF], F32, tag=f"lnr{i}_{h}", name=f"lnr{i}_{h}")
                 for h in range(TN)] for i in range(2)]

        # One arena pool; lifetime groups share tags so SBUF is reused:
        #   G0: xT -> QT -> y      G1: xnT -> attnT -> yT -> hT
        #   G2: wk -> V -> xn2T    G3: KT -> w1    G4: wq -> wv -> wp -> w2
        # (wp/w1/w2 sit in groups that die early so their DMAs prefetch)
        arena = ES.enter_context(tc.tile_pool(name="arena", bufs=1))

        def garr(g, nm, shape=(P, T), dtype=F32):
            return [arena.tile(list(shape), dtype, tag=f"G{g}_{i}",
                               name=f"{nm}{i}") for i in range(KC)]

        def ln_stats_pass(src_loader, ln_m_d, ln_r_d, ph, dst_T=None):
            """For each 128-row chunk j of a [T, C] normal-layout tensor:
            produce the tile, bn_stats -> mean/rstd columns -> DRAM rows,
            and PE-transpose the tile into dst_T chunks (if given)."""
            with ExitStack() as S:
                rows = S.enter_context(tc.tile_pool(name=f"rows{ph}", bufs=4))
                stp = S.enter_context(tc.tile_pool(name=f"stp{ph}", bufs=4))
                pst = S.enter_context(
                    tc.tile_pool(name=f"pst{ph}", bufs=6, space="PSUM"))
                for j in range(TJ):
                    xj = src_loader(rows, j)
                    st = stp.tile([P, 2, 6], F32, tag="st")
                    xr2 = xj[:].rearrange("p (g f) -> p g f", f=NF)
                    for g in range(2):
                        nc.vector.bn_stats(out=st[:, g, :], in_=xr2[:, g, :])
                    mv = stp.tile([P, 2], F32, tag="mv")
                    nc.vector.bn_aggr(out=mv[:], in_=st[:])
                    rs = stp.tile([P, 1], F32, tag="rs")
                    nc.scalar.activation(out=rs[:], in_=mv[:, 1:2],
                                         func=ACTF.Sqrt, bias=epsc[:], scale=1.0)
                    nc.vector.reciprocal(rs[:], rs[:])
                    jh, jo = divmod(j, TJ // TN)
                    nc.sync.dma_start(out=ln_m_d[jh][0, jo * P:(jo + 1) * P],
                                      in_=mv[:, 0:1])
                    nc.sync.dma_start(out=ln_r_d[jh][0, jo * P:(jo + 1) * P],
                                      in_=rs[:])
                    if dst_T is not None:
                        for k in range(KC):
                            pt = pst.tile([P, P], F32, tag="pt")
                            nc.tensor.transpose(pt[:], xj[:, k * P:(k + 1) * P],
                                                ident[:])
                            nc.any.tensor_copy(
                                out=dst_T[k][:, j * P:(j + 1) * P], in_=pt[:])

        def ln_normalize(src_T, dst_T, ln_m_d, ln_r_d, gcol, bcol, ph):
            """dst^T = g*(src^T - mean)*rstd + beta, per t-half."""
            with ExitStack() as S:
                bc = S.enter_context(tc.tile_pool(name=f"bc{ph}", bufs=1))
                for tn in range(TN):
                    tsl = slice(tn * NF, (tn + 1) * NF)
                    mb = bc.tile([P, NF], F32, tag=f"mb{tn}")
                    rb = bc.tile([P, NF], F32, tag=f"rb{tn}")
                    nc.sync.dma_start(
                        out=mb[:], in_=ln_m_d[tn][0:1, :].to_broadcast([P, NF]))
                    nc.sync.dma_start(
                        out=rb[:], in_=ln_r_d[tn][0:1, :].to_broadcast([P, NF]))
                    for k in range(KC):
                        nc.vector.tensor_tensor(dst_T[k][:, tsl],
                                                src_T[k][:, tsl], mb[:],
                                                ALU.subtract)
                        nc.vector.tensor_tensor(dst_T[k][:, tsl],
                                                dst_T[k][:, tsl], rb[:],
                                                ALU.mult)
                        nc.vector.tensor_scalar(
                            out=dst_T[k][:, tsl], in0=dst_T[k][:, tsl],
                            scalar1=gcol[:, k:k + 1], scalar2=bcol[:, k:k + 1],
                            op0=ALU.mult, op1=ALU.add)

        def load_x(rows, j):
            xj = rows.tile([P, C], F32, tag="xrow")
            nc.sync.dma_start(out=xj[:], in_=x_d[j * P:(j + 1) * P, :])
            return xj

        with (tc.For_i(0, hwloop, 1) if hwloop else nullcontext()):
            for _it in range(loop):
                # ---------- Phase 0+1: load x, stats, transpose, LN1 ----------
                xT = garr(0, "xT")
                ln_stats_pass(load_x, ln_m[0], ln_r[0], 0, dst_T=xT)
                xnT = garr(1, "xnT", dtype=F32R)
                ln_normalize(xT, xnT, ln_m[0], ln_r[0],
                             cols["g1"], cols["be1"], 0)

                # ---------------- Phase 2: QKV ----------------
                wq_sb = garr(4, "wq", (P, C), F32R)
                wk_sb = garr(2, "wk", (P, C), F32R)
                QT = garr(0, "QT", (P, T), F32R)
                KT = garr(3, "KT", (P, T), F32R)
                with ExitStack() as S:
                    psq = S.enter_context(
                        tc.tile_pool(name="psq", bufs=3, space="PSUM"))
                    for k in range(KC):
                        nc.sync.dma_start(out=wq_sb[k][:],
                                          in_=wq_d[k * P:(k + 1) * P, :])
                        nc.sync.dma_start(out=wk_sb[k][:],
                                          in_=wk_d[k * P:(k + 1) * P, :])
                    for tn in range(TN):
                        tsl = slice(tn * NF, (tn + 1) * NF)
                        for m in range(KC):
                            pq = psq.tile([P, NF], F32, tag="pq")
                            for k in range(KC):
                                nc.tensor.matmul(
                                    pq[:], lhsT=wq_sb[k][:, m * P:(m + 1) * P],
                                    rhs=xnT[k][:, tsl],
                                    start=(k == 0), stop=(k == KC - 1))
                            nc.any.tensor_copy(out=QT[m][:, tsl], in_=pq[:])
                            pk = psq.tile([P, NF], F32, tag="pk")
                            for k in range(KC):
                                nc.tensor.matmul(
                                    pk[:], lhsT=wk_sb[k][:, m * P:(m + 1) * P],
                                    rhs=xnT[k][:, tsl],
                                    start=(k == 0), stop=(k == KC - 1))
                            nc.any.tensor_copy(out=KT[m][:, tsl], in_=pk[:])

                wv_sb = garr(4, "wv", (P, C), F32R)
                V = garr(2, "V", (P, H, Dh + 1), F32R)
                with ExitStack() as S:
                    psv = S.enter_context(
                        tc.tile_pool(name="psv", bufs=4, space="PSUM"))
                    for k in range(KC):
                        nc.sync.dma_start(out=wv_sb[k][:],
                                          in_=wv_d[k * P:(k + 1) * P, :])
                    for j in range(TJ):
                        nc.vector.tensor_copy(
                            out=V[j][:, :, Dh:Dh + 1],
                            in_=ones1[:, None, 0:1].to_broadcast([P, H, 1]))
                        for hn in range(TN):   # head groups of 8
                            pv = psv.tile([P, NF], F32, tag="pv")
                            for k in range(KC):
                                nc.tensor.matmul(
                                    pv[:], lhsT=xnT[k][:, j * P:(j + 1) * P],
                                    rhs=wv_sb[k][:, hn * NF:(hn + 1) * NF],
                                    start=(k == 0), stop=(k == KC - 1))
                            nc.any.tensor_copy(
                                out=V[j][:, hn * 8:(hn + 1) * 8, 0:Dh],
                                in_=pv[:].rearrange("p (h d) -> p h d", d=Dh))

                # ---------------- Phase 3: attention ----------------
                attnT = garr(1, "attnT", (P, T), F32R)
                with ExitStack() as S:
                    ep = S.enter_context(tc.tile_pool(name="ep", bufs=3))
                    rp = S.enter_context(tc.tile_pool(name="rp", bufs=2))
                    pss = S.enter_context(
                        tc.tile_pool(name="pss", bufs=2, space="PSUM"))
                    pacc = S.enter_context(
                        tc.tile_pool(name="pacc", bufs=2, space="PSUM"))
                    for m in range(KC):
                        h0, h1 = 2 * m, 2 * m + 1
                        for tn in range(TN):
                            tsl = slice(tn * NF, (tn + 1) * NF)
                            pa0 = pacc.tile([65, NF], F32, tag="pa0")
                            pa1 = pacc.tile([65, NF], F32, tag="pa1")
                            i_hi = 4 * (tn + 1)
                            for i in range(i_hi):
                                diag = i - 4 * tn
                                ssl = slice(i * P, (i + 1) * P)
                                ps0 = pss.tile([P, NF], F32, tag="ps0")
                                ps1 = pss.tile([P, NF], F32, tag="ps1")
                                nc.tensor.matmul(ps0[:], lhsT=KT[m][0:64, ssl],
                                                 rhs=QT[m][0:64, tsl],
                                                 start=True, stop=True)
                                nc.tensor.matmul(ps1[:], lhsT=KT[m][64:128, ssl],
                                                 rhs=QT[m][64:128, tsl],
                                                 start=True, stop=True)
                                E0 = ep.tile([P, NF], F32R, tag="E0")
                                E1 = ep.tile([P, NF], F32R, tag="E1")
                                if diag >= 0:
                                    dsl = slice(diag * P, (diag + 1) * P)
                                    nc.vector.tensor_tensor(
                                        ps0[:, dsl], ps0[:, dsl], maskS[:],
                                        ALU.add)
                                    nc.vector.tensor_tensor(
                                        ps1[:, dsl], ps1[:, dsl], maskS[:],
                                        ALU.add)
                                d0_raw = max(diag, 0) * P
                                d0 = min(d0_raw, NF - 256)
                                esl = slice(d0, NF)
                                nc.scalar.activation(
                                    out=E0[:, esl], in_=ps0[:, esl],
                                    func=ACTF.Exp, scale=Dh ** -0.5)
                                nc.scalar.activation(
                                    out=E1[:, esl], in_=ps1[:, esl],
                                    func=ACTF.Exp, scale=Dh ** -0.5)
                                if d0 < d0_raw:
                                    zsl = slice(d0, d0_raw)
                                    zw = d0_raw - d0
                                    nc.vector.tensor_copy(
                                        out=E0[:, zsl],
                                        in_=zeros1[:, 0:1].to_broadcast([P, zw]))
                                    nc.vector.tensor_copy(
                                        out=E1[:, zsl],
                                        in_=zeros1[:, 0:1].to_broadcast([P, zw]))
                                psl = slice(d0, NF)
                                nc.tensor.matmul(
                                    pa0[:, psl], lhsT=V[i][:, h0, :],
                                    rhs=E0[:, psl],
                                    start=(i == 0), stop=(i == i_hi - 1))
                                nc.tensor.matmul(
                                    pa1[:, psl], lhsT=V[i][:, h1, :],
                                    rhs=E1[:, psl],
                                    start=(i == 0), stop=(i == i_hi - 1))
                            # normalize by row 64 (softmax denominator)
                            r0 = rp.tile([1, NF], F32, tag="r0")
                            r1 = rp.tile([1, NF], F32, tag="r1")
                            nc.vector.reciprocal(r0[:], pa0[64:65, :])
                            nc.vector.reciprocal(r1[:], pa1[64:65, :])
                            drs = dram.tile([2, NF], F32, tag="sums")
                            nc.sync.dma_start(out=drs[0:1, :], in_=r0[:])
                            nc.sync.dma_start(out=drs[1:2, :], in_=r1[:])
                            rb0 = rp.tile([64, NF], F32, tag="rb0")
                            rb1 = rp.tile([64, NF], F32, tag="rb1")
                            nc.sync.dma_start(
                                out=rb0[:], in_=drs[0:1, :].to_broadcast([64, NF]))
                            nc.sync.dma_start(
                                out=rb1[:], in_=drs[1:2, :].to_broadcast([64, NF]))
                            nc.vector.tensor_tensor(attnT[m][0:64, tsl],
                                                    pa0[0:64, :], rb0[:],
                                                    ALU.mult)
                            tmp1 = rp.tile([64, NF], F32R, tag="tmp1")
                            nc.vector.tensor_tensor(tmp1[:], pa1[0:64, :],
                                                    rb1[:], ALU.mult)
                            nc.sync.dma_start(out=attnT[m][64:128, tsl],
                                              in_=tmp1[:])

                # ---------- Phase 4: proj + residual -> y (normal) ----------
                wp_sb = garr(4, "wp", (P, C), F32R)
                y_n = garr(0, "y", (P, C), F32)
                with ExitStack() as S:
                    xrp = S.enter_context(tc.tile_pool(name="xrp", bufs=3))
                    psp = S.enter_context(
                        tc.tile_pool(name="psp", bufs=6, space="PSUM"))
                    for k in range(KC):
                        nc.sync.dma_start(out=wp_sb[k][:],
                                          in_=wp_d[k * P:(k + 1) * P, :])
                    for j in range(TJ):
                        xr = xrp.tile([P, C], F32, tag="xr")
                        nc.sync.dma_start(out=xr[:], in_=x_d[j * P:(j + 1) * P, :])
                        for nn in range(TN):
                            csl = slice(nn * NF, (nn + 1) * NF)
                            pp = psp.tile([P, NF], F32, tag="pp")
                            korder = list(range(KC - 1, -1, -1))
                            for ki, k in enumerate(korder):
                                nc.tensor.matmul(
                                    pp[:], lhsT=attnT[k][:, j * P:(j + 1) * P],
                                    rhs=wp_sb[k][:, csl],
                                    start=(ki == 0), stop=(ki == KC - 1))
                            nc.vector.tensor_tensor(y_n[j][:, csl], pp[:],
                                                    xr[:, csl], ALU.add)
                            nc.vector.tensor_tensor(y_n[j][:, csl],
                                                    y_n[j][:, csl],
                                                    bpb[:, csl], ALU.add)

                # ---------------- Phase 5: LN2 ----------------
                yT = garr(1, "yT", (P, T), F32)

                def load_y(rows, j):
                    return y_n[j]

                ln_stats_pass(load_y, ln_m[1], ln_r[1], 1, dst_T=yT)
                xn2T = garr(2, "xn2T", (P, T), F32R)
                ln_normalize(yT, xn2T, ln_m[1], ln_r[1],
                             cols["g2"], cols["be2"], 1)

                # ---------------- Phase 6: MLP fc1 + relu ----------------
                w1_sb = garr(3, "w1", (P, C), F32R)
                hT = garr(1, "hT", (P, T), F32R)
                with ExitStack() as S:
                    psh = S.enter_context(
                        tc.tile_pool(name="psh", bufs=6, space="PSUM"))
                    for k in range(KC):
                        nc.sync.dma_start(out=w1_sb[k][:],
                                          in_=w1_d[k * P:(k + 1) * P, :])
                    for tn in range(TN):
                        tsl = slice(tn * NF, (tn + 1) * NF)
                        for m in range(KC):
                            ph = psh.tile([P, NF], F32, tag="ph")
                            for k in range(KC):
                                nc.tensor.matmul(
                                    ph[:], lhsT=w1_sb[k][:, m * P:(m + 1) * P],
                                    rhs=xn2T[k][:, tsl],
                                    start=(k == 0), stop=(k == KC - 1))
                            nc.scalar.activation(out=hT[m][:, tsl], in_=ph[:],
                                                 func=ACTF.Relu,
                                                 bias=cols["b1"][:, m:m + 1],
                                                 scale=1.0)

                # ---------- Phase 7: MLP fc2 + residual -> out ----------
                w2_sb = garr(4, "w2", (P, C), F32R)
                with ExitStack() as S:
                    otp = S.enter_context(tc.tile_pool(name="otp", bufs=3))
                    pso = S.enter_context(
                        tc.tile_pool(name="pso", bufs=6, space="PSUM"))
                    for k in range(KC):
                        nc.sync.dma_start(out=w2_sb[k][:],
                                          in_=w2_d[k * P:(k + 1) * P, :])
                    for j in range(TJ):
                        for nn in range(TN):
                            csl = slice(nn * NF, (nn + 1) * NF)
                            po = pso.tile([P, NF], F32, tag="po")
                            korder = list(range(KC - 1, -1, -1))
                            for ki, k in enumerate(korder):
                                nc.tensor.matmul(
                                    po[:], lhsT=hT[k][:, j * P:(j + 1) * P],
                                    rhs=w2_sb[k][:, csl],
                                    start=(ki == 0), stop=(ki == KC - 1))
                            ot = otp.tile([P, NF], F32, tag="ot")
                            nc.vector.tensor_tensor(ot[:], po[:],
                                                    y_n[j][:, csl], ALU.add)
                            nc.vector.tensor_tensor(ot[:], ot[:], b2b[:, csl],
                                                    ALU.add)
                            nc.sync.dma_start(
                                out=out_d[j * P:(j + 1) * P, csl], in_=ot[:])

    nc.compile()
    return nc


def _prep_inputs(inputs):
    """Host-side weight repacking; returns per-core in_maps."""
    f = np.float32
    x = np.ascontiguousarray(np.asarray(inputs["x"], dtype=f))        # [B, T, C]
    wq = np.ascontiguousarray(
        np.asarray(inputs["Wq"], dtype=f).transpose(1, 0, 2).reshape(C, C))
    wk = np.ascontiguousarray(
        np.asarray(inputs["Wk"], dtype=f).transpose(1, 0, 2).reshape(C, C))
    wv = np.ascontiguousarray(
        np.asarray(inputs["Wv"], dtype=f).transpose(1, 0, 2).reshape(C, C))
    common = {
        "wq": wq, "wk": wk, "wv": wv,
        "wp": np.ascontiguousarray(np.asarray(inputs["Wproj"], dtype=f)),
        "w1": np.ascontiguousarray(np.asarray(inputs["W1"], dtype=f)),
        "w2": np.ascontiguousarray(np.asarray(inputs["W2"], dtype=f)),
        "bp": np.asarray(inputs["bproj"], dtype=f),
        "b1": np.asarray(inputs["b1"], dtype=f),
        "b2": np.asarray(inputs["b2"], dtype=f),
        "g1": np.asarray(inputs["g1"], dtype=f),
        "be1": np.asarray(inputs["beta1"], dtype=f),
        "g2": np.asarray(inputs["g2"], dtype=f),
        "be2": np.asarray(inputs["beta2"], dtype=f),
    }
    return [{"x": x[b], **common} for b in range(N_CORES)]


def kernel(**inputs) -> np.ndarray:
    if "nc" not in _CACHE:
        _CACHE["nc"] = build_nc()
    nc = _CACHE["nc"]
    in_maps = _prep_inputs(inputs)
    res = run_bass_kernel_spmd(nc, in_maps, list(range(N_CORES)))
    out = np.stack([res.results[b]["out"] for b in range(N_CORES)], axis=0)
    return out.astype(np.float32)


if __name__ == "__main__":
    rng = np.random.default_rng(0)
    demo = {
        "x": rng.standard_normal((B, T, C), dtype=np.float32),
        "Wq": rng.standard_normal((H, C, Dh), dtype=np.float32) * 0.02,
        "Wk": rng.standard_normal((H, C, Dh), dtype=np.float32) * 0.02,
        "Wv": rng.standard_normal((H, C, Dh), dtype=np.float32) * 0.02,
        "Wproj": rng.standard_normal((C, C), dtype=np.float32) * 0.02,
        "bproj": np.zeros(C, np.float32),
        "W1": rng.standard_normal((C, C), dtype=np.float32) * 0.02,
        "b1": np.zeros(C, np.float32),
        "W2": rng.standard_normal((C, C), dtype=np.float32) * 0.02,
        "b2": np.zeros(C, np.float32),
        "g1": np.ones(C, np.float32),
        "beta1": np.zeros(C, np.float32),
        "g2": np.ones(C, np.float32),
        "beta2": np.zeros(C, np.float32),
    }
    y = kernel(**demo)
    print("out", y.shape, y.dtype, float(np.abs(y).max()))
